# revision 1
# baseline (speedup 1.0000x reference)
"""BiasAttention Trainium2 Bass kernel.

Computes, for x:[B,Q,CV], bias1:[B,H,Q,Q], bias2:[B,1,Q,Q], W_v/W_g:[CV,H*CH],
W_o:[H*CH,CV]:

    v = (x @ W_v) viewed [B,Q,H,CH]
    a = softmax(bias1 + bias2, axis=-1)
    o = einsum('bhqk,bhkd->bhqd', a, v) * sigmoid(x @ W_g)
    return o @ W_o

Sharding: data-parallel over the query dim Q across 8 NeuronCores (each core
computes 256 query rows end-to-end; no collectives). bias1/bias2 are
transposed on the host while slicing per core ([B,H,Q,QL]) so the kernel
loads the softmax operand with k already on partitions. Per core:
  - DMA b1T tiles [128, kg*QL] (3-D AP, 1KB elements, full bandwidth)
  - z = b1T + b2T on VectorE (fp32)
  - s = exp(z) on ScalarE SBUF->SBUF (fp32r out)
  - oT[d,q] (+ row-sum via a ones column packed at stride 65 in the v tile)
    accumulated on TensorE with fp32r matmuls (1 cyc/row at free dim >= 256)
  - gate = sigmoid(x_loc @ W_g) * (1/rowsum), applied transposed; final
    projection @ W_o per 128-row q tile.
v = x @ W_v is computed per core (x transposed on TensorE), interleaved into
the first head's key-groups.
"""

import contextlib

import numpy as np


def _ensure_concourse():
    try:
        import concourse  # noqa: F401
    except ImportError:
        import sys

        for p in ("/root/.axon_site/_ro/trn_rl_repo", "/opt/trn_rl_repo"):
            if p not in sys.path:
                sys.path.insert(0, p)


_ensure_concourse()

import concourse.bacc as bacc  # noqa: E402
import concourse.mybir as mybir  # noqa: E402
import concourse.tile as tile  # noqa: E402
from concourse import bass_utils  # noqa: E402

F32 = mybir.dt.float32
F32R = mybir.dt.float32r
AF = mybir.ActivationFunctionType

# Problem dims (nn_BiasAttention): hardcoded per the harness contract.
CFG = dict(B=2, Q=2048, CV=512, H=8, CH=64, NCORES=8)


def build(cfg=None, repeat=1, kg=4, wu=1, psz=2, pso=3, psav=3, b1b=10, stb=4, xtb=2, ablate=()):
    """Build the per-core Bass program (identical on all cores; sharding is
    done by host-side input slicing). repeat>1 wraps the body in a hardware
    loop for timing runs. kg = key-blocks per exp group."""
    c = dict(CFG if cfg is None else cfg)
    B, Q, CV, H, CH, NCORES = c["B"], c["Q"], c["CV"], c["H"], c["CH"], c["NCORES"]
    HD = H * CH
    QL = Q // NCORES  # query rows per core
    QT = QL // 128  # q tiles per core
    KB = Q // 128  # key blocks
    CVB = CV // 128
    HDB = HD // 128
    JL = B * QL // 128  # x row-tiles (local, for the gate)
    DH1 = CH + 1  # head dim + ones column (row-sum trick)
    assert QL % 128 == 0 and CV % 128 == 0 and HD % 128 == 0 and CH == 64
    if kg is None:
        kg = min(8, KB)
    if kg * wu > KB:
        wu = KB // kg
    assert H % 2 == 0 and KB % (kg * wu) == 0

    nc = bacc.Bacc("TRN2", target_bir_lowering=False, debug=False, num_devices=NCORES)

    x_d = nc.dram_tensor("x", [B * Q, CV], F32, kind="ExternalInput")
    xl_d = nc.dram_tensor("xl", [B * QL, CV], F32, kind="ExternalInput")
    b1_d = nc.dram_tensor("b1", [B, H, Q, QL], F32, kind="ExternalInput")
    b2_d = nc.dram_tensor("b2", [B, Q, QL], F32, kind="ExternalInput")
    wv_d = nc.dram_tensor("wv", [CV, HD], F32, kind="ExternalInput")
    wg_d = nc.dram_tensor("wg", [CV, HD], F32, kind="ExternalInput")
    wo_d = nc.dram_tensor("wo", [HD, CV], F32, kind="ExternalInput")
    id_d = nc.dram_tensor("ident", [128, 128], F32, kind="ExternalInput")
    ones_d = nc.dram_tensor("ones", [128, KB * H], F32, kind="ExternalInput")
    out_d = nc.dram_tensor("out", [B, QL, CV], F32, kind="ExternalOutput")

    with tile.TileContext(nc) as tc:
        loop = tc.For_i(0, repeat, 1) if repeat > 1 else contextlib.nullcontext()
        with loop, contextlib.ExitStack() as ctx:
            persist = ctx.enter_context(tc.tile_pool(name="persist", bufs=1))
            b1p = ctx.enter_context(tc.tile_pool(name="b1p", bufs=b1b))
            xp = ctx.enter_context(tc.tile_pool(name="xp", bufs=4))
            xTp = ctx.enter_context(tc.tile_pool(name="xTp", bufs=xtb))
            sTp = ctx.enter_context(tc.tile_pool(name="sTp", bufs=stb))
            psAV = ctx.enter_context(tc.tile_pool(name="psAV", bufs=psav, space="PSUM"))
            psO = ctx.enter_context(tc.tile_pool(name="psO", bufs=pso, space="PSUM"))

            # ---- persistent tiles ----
            id32 = persist.tile([128, 128], F32, name="id32", tag="id32")
            idr = persist.tile([128, 128], F32R, name="idr", tag="idr")
            wv_t = persist.tile([128, CVB * HD], F32R, name="wv_t", tag="wv_t")
            wg_t = persist.tile([128, CVB * HD], F32R, name="wg_t", tag="wg_t")
            wo_t = persist.tile([128, HDB * CV], F32R, name="wo_t", tag="wo_t")
            v_aug = [
                persist.tile([128, KB * H * DH1], F32R, name=f"vaug{b}", tag=f"vaug{b}")
                for b in range(B)
            ]
            b2_t = [
                persist.tile([128, KB * QL], F32, name=f"b2_{b}", tag=f"b2_{b}")
                for b in range(B)
            ]
            g_sb = [
                persist.tile([128, HD], F32R, name=f"g_{jl}", tag=f"g_{jl}")
                for jl in range(JL)
            ]
            ogT = [
                persist.tile([128, HDB * QL], F32R, name=f"ogT{b}", tag=f"ogT{b}")
                for b in range(B)
            ]
            rs_sb = [
                persist.tile([H, QL], F32, name=f"rs{b}", tag="rs") for b in range(B)
            ]
            recip = [
                persist.tile([128, QT * H], F32, name=f"rcp{b}", tag="rcp")
                for b in range(B)
            ]

            # bias2 first: it gates the very first z-add
            for b in range(B):
                nc.scalar.dma_start(
                    b2_t[b][:].rearrange("p (kb q) -> p kb q", q=QL),
                    b2_d[b].rearrange("(kb p) q -> p kb q", p=128),
                )
            nc.scalar.dma_start(id32[:], id_d[:])
            nc.vector.tensor_copy(idr[:], id32[:])
            for w_t, w_d, nblk, dcol in (
                (wv_t, wv_d, CVB, HD),
                (wg_t, wg_d, CVB, HD),
                (wo_t, wo_d, HDB, CV),
            ):
                for cb in range(nblk):
                    wstage = xp.tile([128, dcol], F32, name="wstage", tag="xt")
                    nc.scalar.dma_start(wstage[:], w_d[cb * 128 : (cb + 1) * 128, :])
                    nc.vector.tensor_copy(
                        w_t[:, cb * dcol : (cb + 1) * dcol], wstage[:]
                    )

            # ones columns of v_aug (row-sum trick): contiguous DMA then
            # one strided DVE copy per batch
            ones_sb = persist.tile([128, KB * H], F32, name="ones_sb", tag="ones_sb")
            nc.scalar.dma_start(ones_sb[:], ones_d[:])
            for b in range(B):
                ones_ap = v_aug[b][:].rearrange("p (n d) -> p n d", d=DH1)[:, :, CH]
                nc.vector.tensor_copy(ones_ap, ones_sb[:])

            def stage_b_tile(b, kt):
                """one v = x @ W_v row-tile (full Q, replicated on all cores)."""
                if "stageb" in ablate:
                    return
                if True:
                    j = b * KB + kt
                    xt = xp.tile([128, CV], F32, name="xt", tag="xt")
                    nc.scalar.dma_start(xt[:], x_d[j * 128 : (j + 1) * 128, :])
                    xT_ps = psAV.tile([128, CV], F32, name="xT_ps", tag="ps512")
                    for cb in range(CVB):
                        nc.tensor.transpose(
                            xT_ps[:, cb * 128 : (cb + 1) * 128],
                            xt[:, cb * 128 : (cb + 1) * 128],
                            id32[:],
                        )
                    xT = xTp.tile([128, CV], F32R, name="xT", tag="xT")
                    nc.scalar.copy(xT[:], xT_ps[:])
                    v_ps = psAV.tile([128, HD], F32, name="v_ps", tag="ps512")
                    for cb in range(CVB):
                        nc.tensor.matmul(
                            v_ps[:],
                            xT[:, cb * 128 : (cb + 1) * 128],
                            wv_t[:, cb * HD : (cb + 1) * HD],
                            start=(cb == 0),
                            stop=(cb == CVB - 1),
                        )
                    dst = v_aug[b][:].rearrange("p (kt h d) -> p kt h d", h=H, d=DH1)[
                        :, kt, :, 0:CH
                    ]
                    src = v_ps[:].rearrange("p (h d) -> p h d", d=CH)
                    nc.vector.tensor_copy(dst, src)

            def stage_g(b):
                """gate inputs g = sigmoid(x_loc @ W_g) for batch b."""
                if "stageb" in ablate:
                    return
                for qt in range(QT):
                    jl = b * QT + qt
                    xt = xp.tile([128, CV], F32, name="xt", tag="xt")
                    nc.scalar.dma_start(xt[:], xl_d[jl * 128 : (jl + 1) * 128, :])
                    xT_ps = psAV.tile([128, CV], F32, name="xT_ps", tag="ps512")
                    for cb in range(CVB):
                        nc.tensor.transpose(
                            xT_ps[:, cb * 128 : (cb + 1) * 128],
                            xt[:, cb * 128 : (cb + 1) * 128],
                            id32[:],
                        )
                    xT = xTp.tile([128, CV], F32R, name="xT", tag="xT")
                    nc.scalar.copy(xT[:], xT_ps[:])
                    g_ps = psAV.tile([128, HD], F32, name="g_ps", tag="ps512")
                    for cb in range(CVB):
                        nc.tensor.matmul(
                            g_ps[:],
                            xT[:, cb * 128 : (cb + 1) * 128],
                            wg_t[:, cb * HD : (cb + 1) * HD],
                            start=(cb == 0),
                            stop=(cb == CVB - 1),
                        )
                    nc.scalar.activation(g_sb[jl][:], g_ps[:], AF.Sigmoid)

            # ---- main per-batch flow ----
            for b in range(B):
                for h in range(H):
                    oT_ps = psO.tile([DH1, QL], F32, name="oT_ps", tag="psO")
                    kw = kg * wu  # key-blocks per wide add/exp unit
                    for kbu in range(KB // kw):
                        if h == 0:
                            for kt in range(kbu * kw, (kbu + 1) * kw):
                                stage_b_tile(b, kt)
                        b1t = b1p.tile([128, kw * QL], F32, name="b1t", tag="b1t")
                        if "b1dma" not in ablate:
                            for d in range(wu):
                                kbg = kbu * wu + d
                                nc.sync.dma_start(
                                    b1t[:, d * kg * QL : (d + 1) * kg * QL].rearrange(
                                        "p (kb q) -> p kb q", q=QL
                                    ),
                                    b1_d[
                                        b, h, kbg * kg * 128 : (kbg + 1) * kg * 128, :
                                    ].rearrange("(kb p) q -> p kb q", p=128),
                                )
                        zt = b1p.tile([128, kw * QL], F32, name="zt", tag="b1t")
                        if "zadd" not in ablate:
                            nc.vector.tensor_add(
                                zt[:],
                                b1t[:],
                                b2_t[b][:, kbu * kw * QL : (kbu + 1) * kw * QL],
                            )
                        sT = sTp.tile([128, kw * QL], F32R, name="sT", tag="sT")
                        if "exp" not in ablate:
                            nc.scalar.activation(sT[:], zt[:], AF.Exp)
                        if "attn" not in ablate:
                            for kbi in range(kw):
                                kb = kbu * kw + kbi
                                base = (kb * H + h) * DH1
                                nc.tensor.matmul(
                                    oT_ps[:],
                                    v_aug[b][:, base : base + DH1],
                                    sT[:, kbi * QL : (kbi + 1) * QL],
                                    start=(kb == 0),
                                    stop=(kb == KB - 1),
                                )
                    if h == 0:
                        stage_g(b)
                    # epilogue: stash oT (head-paired layout) + row-sums
                    hp, hq = divmod(h, 2)
                    if "epi" in ablate:
                        continue
                    oT_sb = xTp.tile([DH1, QL], F32, name="oT_sb", tag="xT")
                    nc.scalar.copy(oT_sb[:], oT_ps[:])
                    nc.scalar.dma_start(
                        ogT[b][64 * hq : 64 * hq + 64, hp * QL : (hp + 1) * QL],
                        oT_sb[:].bitcast(F32R)[0:64, :],
                    )
                    nc.scalar.dma_start(rs_sb[b][h : h + 1, :], oT_sb[64:65, :])

                # per-b epilogue: reciprocal row-sums, gate, project
                if "epi" in ablate:
                    continue
                rsT_ps = psAV.tile([128, QT * H], F32, name="rsT_ps", tag="ps512")
                for qt in range(QT):
                    nc.tensor.transpose(
                        rsT_ps[:, qt * H : (qt + 1) * H],
                        rs_sb[b][0:H, qt * 128 : (qt + 1) * 128],
                        id32[0:H, 0:H],
                    )
                nc.vector.reciprocal(recip[b][:], rsT_ps[:])
                for qt in range(QT):
                    jl = b * QT + qt
                    gq = g_sb[jl]
                    for h in range(H):
                        col = recip[b][:, qt * H + h : qt * H + h + 1]
                        nc.vector.tensor_scalar_mul(
                            gq[:, h * CH : (h + 1) * CH],
                            gq[:].bitcast(F32)[:, h * CH : (h + 1) * CH],
                            col,
                        )
                    for cb in range(HDB):
                        gT_ps = psAV.tile([128, 128], F32R, name="gT_ps", tag="ps512")
                        nc.tensor.transpose(
                            gT_ps[:],
                            gq[:, cb * 128 : (cb + 1) * 128],
                            idr[:],
                        )
                        dst = ogT[b][:, cb * QL + qt * 128 : cb * QL + qt * 128 + 128]
                        nc.vector.tensor_mul(
                            dst, dst.bitcast(F32), gT_ps[:].bitcast(F32)
                        )
                    o_ps = psAV.tile([128, CV], F32, name="o_ps", tag="ps512")
                    for cb in range(HDB):
                        nc.tensor.matmul(
                            o_ps[:],
                            ogT[b][:, cb * QL + qt * 128 : cb * QL + qt * 128 + 128],
                            wo_t[:, cb * CV : (cb + 1) * CV],
                            start=(cb == 0),
                            stop=(cb == HDB - 1),
                        )
                    o_sb = xp.tile([128, CV], F32, name="o_sb", tag="xt")
                    nc.scalar.copy(o_sb[:], o_ps[:])
                    nc.scalar.dma_start(out_d[b, qt * 128 : (qt + 1) * 128, :], o_sb[:])

    nc.compile()
    return nc


def make_in_maps(inputs, cfg=None):
    c = dict(CFG if cfg is None else cfg)
    B, Q, CV, NCORES, H = c["B"], c["Q"], c["CV"], c["NCORES"], c["H"]
    QL = Q // NCORES
    KB = Q // 128
    x = np.ascontiguousarray(np.asarray(inputs["x"], dtype=np.float32)).reshape(
        B * Q, CV
    )
    b1 = np.asarray(inputs["bias1"], dtype=np.float32)
    b2 = np.asarray(inputs["bias2"], dtype=np.float32)
    ident = np.eye(128, dtype=np.float32)
    wv = np.ascontiguousarray(np.asarray(inputs["W_v"], dtype=np.float32))
    wg = np.ascontiguousarray(np.asarray(inputs["W_g"], dtype=np.float32))
    wo = np.ascontiguousarray(np.asarray(inputs["W_o"], dtype=np.float32))
    xr = np.asarray(inputs["x"], dtype=np.float32)
    in_maps = []
    for cid in range(NCORES):
        sl = slice(cid * QL, (cid + 1) * QL)
        in_maps.append(
            {
                "x": x,
                "xl": np.ascontiguousarray(xr[:, sl, :]).reshape(B * QL, CV),
                "b1": np.ascontiguousarray(b1[:, :, sl, :].swapaxes(2, 3)),
                "b2": np.ascontiguousarray(b2[:, 0, sl, :].swapaxes(1, 2)),
                "wv": wv,
                "wg": wg,
                "wo": wo,
                "ident": ident,
                "ones": np.ones((128, KB * H), dtype=np.float32),
            }
        )
    return in_maps


_NC_CACHE = {}


def kernel(**inputs) -> np.ndarray:
    key = "main"
    if key not in _NC_CACHE:
        _NC_CACHE[key] = build()
    nc = _NC_CACHE[key]
    in_maps = make_in_maps(inputs)
    res = bass_utils.run_bass_kernel_spmd(nc, in_maps, list(range(CFG["NCORES"])))
    outs = [res.results[cid]["out"] for cid in range(CFG["NCORES"])]
    return np.concatenate(outs, axis=1).astype(np.float32)

